# revision 10
# baseline (speedup 1.0000x reference)
"""AssocScan Trainium2 kernel: out[:, t] = gates[:, t] * out[:, t-1] + inputs[:, t].

The recurrence is independent per (b, d) lane (B*D = 4096 lanes, N = 4096
steps); 512 lanes per core, 4 per SBUF partition. The DVE
`tensor_tensor_scan` runs at a measured ~2.14 ns/column and is the only
engine with the scan opcode (gpsimd is rejected by the ISA check), so a
pure scan kernel floors at ~35.4 us of DVE time.

Hybrid strategy (measured rates: DVE tensor_tensor bf16 0.56 ns/col,
gpsimd tensor_tensor bf16 2.38 ns/col):
 - 2 of the 4 lanes per partition are scanned directly (plain stream,
   [128, 8192]).
 - The other 2 lanes are pair-unrolled: h[2k+1] = A[k]*h[2k-1] + B[k]
   with A = g_odd*g_even, B = g_odd*x_even + x_odd. The gpsimd engine
   computes A, T = g_odd*x_even, B = T + x_odd (3 passes, ~29 us total),
   the DVE scans the half-length pair stream ([128, 4096], ~8.8 us) and
   expands the even positions (h[2k] = g_even*H[k-1] + x_even, 2 fast
   elementwise passes, ~4.6 us).
DVE total ~30.9 us and gpsimd ~29.3 us run concurrently, beating the
35.4 us pure-scan floor; DMA (~12.6 MB at ~400+ B/ns) overlaps both.

Because the host zeroes g[:, 0] of every lane (it multiplies the zero
initial state), the scan state self-resets at lane boundaries, so both
streams chain across chunk boundaries through the previous chunk's last
output column, and the even-expansion of a lane's first pair sees
g_even = 0, killing any stale carry.

Layout/DMA: per-ring HWDGE throughput is descriptor-size bound (~130 B/ns
at 1 KB lines up to ~350 at 8 KB), so the pair-stream ships g_even||g_odd
packed in one DRAM tensor per chunk (double lines, one transfer). g-side
rides the sync ring, x-side the scalar ring, in consumption order; stores
queue behind the loads on both rings.
"""

import sys

import numpy as np

for _p in ("/opt/trn_rl_repo", "/opt/pypackages"):
    if _p not in sys.path:
        sys.path.append(_p)

import concourse.bacc as bacc
import concourse.mybir as mybir
from concourse.bass_utils import run_bass_kernel_spmd
from concourse.tile import TileContext

B, N, D = 4, 4096, 1024
N_CORES = 8
LANES = B * D                        # 4096 independent (b, d) lanes
LANES_PER_CORE = LANES // N_CORES    # 512
P = 128                              # SBUF partitions
LPP = LANES_PER_CORE // P            # 4 lanes per partition
NP_COLS = 2 * N                      # plain stream columns per partition (2 lanes)
NU_COLS = N                          # pair-stream columns per partition (2 lanes / 2)

TRACE = False
USE_BF16 = True     # kept for test.py compatibility; bf16 is required for rate
BF16_OUT = True
_result_info = {}

import os as _os

# Plain-stream scan chunks (sum = 8192) and pair-stream chunks (sum = 4096).
_PCH = [int(s) for s in _os.environ.get("PCH", "512,1024,2048,4608").split(",")]
_UCH = [int(s) for s in _os.environ.get("UCH", "1280,1280,1024,512").split(",")]
assert sum(_PCH) == NP_COLS and sum(_UCH) == NU_COLS
# DVE program order: interleave plain chunks and pair-chunk work so the DVE
# stays busy while gpsimd prepares the next pair chunk. Entries: ("p", i) or
# ("u", c).
_ORDER = [("p", 0), ("p", 1), ("p", 2), ("u", 0), ("p", 3), ("u", 1), ("u", 2), ("u", 3)]
if _os.environ.get("ORDER"):
    _ORDER = [
        (t[0], int(t[1:])) for t in _os.environ["ORDER"].split(",")
    ]


def _build() -> bacc.Bacc:
    bf = mybir.dt.bfloat16
    nc = bacc.Bacc()
    M = mybir.AluOpType.mult
    A_ = mybir.AluOpType.add

    gps = [nc.dram_tensor(f"gp{k}", [P, s], bf, kind="ExternalInput") for k, s in enumerate(_PCH)]
    xps = [nc.dram_tensor(f"xp{k}", [P, s], bf, kind="ExternalInput") for k, s in enumerate(_PCH)]
    ops = [nc.dram_tensor(f"op{k}", [P, s], bf, kind="ExternalOutput") for k, s in enumerate(_PCH)]
    # pair stream: g_even||g_odd packed per chunk, likewise x
    gus = [nc.dram_tensor(f"gu{c}", [P, 2 * s], bf, kind="ExternalInput") for c, s in enumerate(_UCH)]
    xus = [nc.dram_tensor(f"xu{c}", [P, 2 * s], bf, kind="ExternalInput") for c, s in enumerate(_UCH)]
    oss = [nc.dram_tensor(f"os{c}", [P, s], bf, kind="ExternalOutput") for c, s in enumerate(_UCH)]
    oes = [nc.dram_tensor(f"oe{c}", [P, s], bf, kind="ExternalOutput") for c, s in enumerate(_UCH)]

    with TileContext(nc) as tc:
        with tc.tile_pool(name="pool", bufs=1) as pool:
            gpt = [pool.tile([P, s], bf, name=f"gpt{k}") for k, s in enumerate(_PCH)]
            xpt = [pool.tile([P, s], bf, name=f"xpt{k}") for k, s in enumerate(_PCH)]
            opt = [pool.tile([P, s], bf, name=f"opt{k}") for k, s in enumerate(_PCH)]
            gut = [pool.tile([P, 2 * s], bf, name=f"gut{c}") for c, s in enumerate(_UCH)]
            xut = [pool.tile([P, 2 * s], bf, name=f"xut{c}") for c, s in enumerate(_UCH)]
            at = [pool.tile([P, s], bf, name=f"at{c}") for c, s in enumerate(_UCH)]
            tt = [pool.tile([P, s], bf, name=f"tt{c}") for c, s in enumerate(_UCH)]
            bt = [pool.tile([P, s], bf, name=f"bt{c}") for c, s in enumerate(_UCH)]
            # S tile has one extra leading column holding the previous
            # chunk's last H so the shifted read Hm1 = st[:, 0:s] works.
            st = [pool.tile([P, s + 1], bf, name=f"st{c}") for c, s in enumerate(_UCH)]
            et = [pool.tile([P, s], bf, name=f"et{c}") for c, s in enumerate(_UCH)]
            oet = [pool.tile([P, s], bf, name=f"oet{c}") for c, s in enumerate(_UCH)]

            # Loads in consumption order; g-side on sync, x-side on scalar.
            # The pair chunk c is consumed (by gpsimd) earlier than most plain
            # chunks, so interleave: p0, u0, p1, u1, p2, u2, p3, u3.
            load_order = []
            np_, nu_ = len(_PCH), len(_UCH)
            for i in range(max(np_, nu_)):
                if i < np_:
                    load_order.append(("p", i))
                if i < nu_:
                    load_order.append(("u", i))
            for kind, i in load_order:
                if kind == "p":
                    nc.sync.dma_start(out=gpt[i][:, :], in_=gps[i][:, :])
                    nc.scalar.dma_start(out=xpt[i][:, :], in_=xps[i][:, :])
                else:
                    nc.sync.dma_start(out=gut[i][:, :], in_=gus[i][:, :])
                    nc.scalar.dma_start(out=xut[i][:, :], in_=xus[i][:, :])

            # gpsimd: seed chunk 0's leading H column (no-read constant
            # write; avoids NaN garbage reaching 0*H at lane starts), then
            # the pair aggregates, serial chain chunk by chunk.
            nc.gpsimd.memset(st[0][:, 0:1], 0.0)
            for c, s in enumerate(_UCH):
                ge, go = gut[c][:, 0:s], gut[c][:, s : 2 * s]
                xe, xo = xut[c][:, 0:s], xut[c][:, s : 2 * s]
                nc.gpsimd.tensor_tensor(at[c][:, :], go, ge, M)
                nc.gpsimd.tensor_tensor(tt[c][:, :], go, xe, M)
                nc.gpsimd.tensor_tensor(bt[c][:, :], tt[c][:, :], xo, A_)

            # Scalar engine: chunk c's leading H column = previous chunk's
            # last H (scalar is idle; a 1-col activation copy per chunk).
            for c in range(1, len(_UCH)):
                nc.scalar.copy(out=st[c][:, 0:1], in_=st[c - 1][:, _UCH[c - 1] : _UCH[c - 1] + 1])

            # DVE: plain scans + pair scans/expansions, in _ORDER.
            prev_p = None
            prev_u = None
            for kind, i in _ORDER:
                if kind == "p":
                    s = _PCH[i]
                    init = 0.0 if prev_p is None else prev_p
                    nc.vector.tensor_tensor_scan(
                        opt[i][:, :], gpt[i][:, :], xpt[i][:, :], init, M, A_
                    )
                    prev_p = opt[i][:, s - 1 : s]
                    nc.sync.dma_start(out=ops[i][:, :], in_=opt[i][:, :])
                else:
                    s = _UCH[i]
                    ge = gut[i][:, 0:s]
                    xe = xut[i][:, 0:s]
                    init = 0.0 if prev_u is None else prev_u
                    nc.vector.tensor_tensor_scan(
                        st[i][:, 1 : s + 1], at[i][:, :], bt[i][:, :], init, M, A_
                    )
                    prev_u = st[i][:, s : s + 1]
                    # h_even = g_even * H[k-1] + x_even
                    nc.vector.tensor_tensor(et[i][:, :], ge, st[i][:, 0:s], M)
                    nc.vector.tensor_tensor(oet[i][:, :], et[i][:, :], xe, A_)
                    nc.sync.dma_start(out=oss[i][:, :], in_=st[i][:, 1 : s + 1])
                    nc.scalar.dma_start(out=oes[i][:, :], in_=oet[i][:, :])
    nc.compile()
    return nc


def kernel(gates: np.ndarray, inputs: np.ndarray) -> np.ndarray:
    import ml_dtypes

    gates = np.asarray(gates, dtype=np.float32)
    inputs = np.asarray(inputs, dtype=np.float32)

    # (B, N, D) -> lane-major (B*D, N); row b*D + d is the time series of
    # lane (b, d). First gate of every lane is dead (multiplies zero state).
    gt = np.ascontiguousarray(gates.transpose(0, 2, 1)).reshape(LANES, N)
    xt = np.ascontiguousarray(inputs.transpose(0, 2, 1)).reshape(LANES, N)
    gt[:, 0] = 0.0
    gt = gt.astype(ml_dtypes.bfloat16)
    xt = xt.astype(ml_dtypes.bfloat16)

    pb = np.cumsum([0] + _PCH)
    ub = np.cumsum([0] + _UCH)
    in_maps = []
    for c in range(N_CORES):
        rows = slice(c * LANES_PER_CORE, (c + 1) * LANES_PER_CORE)
        # [512, N] -> [LPP, P, N]; partition p holds lanes {base+p, +128, ...}
        gc = gt[rows].reshape(LPP, P, N)
        xc = xt[rows].reshape(LPP, P, N)
        # plain stream: lanes 0,1 -> [P, 2N]
        gpl = gc[0:2].transpose(1, 0, 2).reshape(P, NP_COLS)
        xpl = xc[0:2].transpose(1, 0, 2).reshape(P, NP_COLS)
        # pair stream: lanes 2,3, even/odd split -> [P, N] each
        gue = gc[2:4, :, 0::2].transpose(1, 0, 2).reshape(P, NU_COLS)
        guo = gc[2:4, :, 1::2].transpose(1, 0, 2).reshape(P, NU_COLS)
        xue = xc[2:4, :, 0::2].transpose(1, 0, 2).reshape(P, NU_COLS)
        xuo = xc[2:4, :, 1::2].transpose(1, 0, 2).reshape(P, NU_COLS)
        m = {}
        for k in range(len(_PCH)):
            sl = slice(pb[k], pb[k + 1])
            m[f"gp{k}"] = np.ascontiguousarray(gpl[:, sl])
            m[f"xp{k}"] = np.ascontiguousarray(xpl[:, sl])
        for k in range(len(_UCH)):
            sl = slice(ub[k], ub[k + 1])
            m[f"gu{k}"] = np.ascontiguousarray(
                np.concatenate([gue[:, sl], guo[:, sl]], axis=1)
            )
            m[f"xu{k}"] = np.ascontiguousarray(
                np.concatenate([xue[:, sl], xuo[:, sl]], axis=1)
            )
        in_maps.append(m)

    nc = _build()
    res = run_bass_kernel_spmd(
        nc, in_maps, core_ids=list(range(N_CORES)), trace=TRACE
    )
    _result_info["exec_time_ns"] = res.exec_time_ns
    _result_info["mean_exec_time_ns"] = res.mean_exec_time_ns
    _result_info["profile_json"] = res.profile_json
    _result_info["trace"] = (
        res.instructions_and_trace[1] if res.instructions_and_trace else None
    )

    out_t = np.empty((LANES, N), dtype=np.float32)
    for c in range(N_CORES):
        r = res.results[c]
        opl = np.concatenate(
            [r[f"op{k}"].astype(np.float32) for k in range(len(_PCH))], axis=1
        )  # [P, 2N]
        hs = np.concatenate(
            [r[f"os{k}"].astype(np.float32) for k in range(len(_UCH))], axis=1
        )  # [P, N] odd positions
        he = np.concatenate(
            [r[f"oe{k}"].astype(np.float32) for k in range(len(_UCH))], axis=1
        )  # [P, N] even positions
        base = c * LANES_PER_CORE
        # plain lanes 0,1
        pl = opl.reshape(P, 2, N).transpose(1, 0, 2)  # [2, P, N]
        out_t[base : base + 2 * P] = pl.reshape(2 * P, N)
        # unrolled lanes 2,3: interleave even/odd
        ue = he.reshape(P, 2, N // 2).transpose(1, 0, 2)  # [2, P, N/2]
        uo = hs.reshape(P, 2, N // 2).transpose(1, 0, 2)
        ul = np.empty((2, P, N), dtype=np.float32)
        ul[:, :, 0::2] = ue
        ul[:, :, 1::2] = uo
        out_t[base + 2 * P : base + 4 * P] = ul.reshape(2 * P, N)
    return np.ascontiguousarray(out_t.reshape(B, D, N).transpose(0, 2, 1))


# revision 11
# speedup vs baseline: 1.4813x; 1.4813x over previous
"""AssocScan Trainium2 kernel: out[:, t] = gates[:, t] * out[:, t-1] + inputs[:, t].

Strategy: the recurrence is independent per (b, d) lane (B*D = 4096 lanes,
N = 4096 steps). The DVE `tensor_tensor_scan` instruction computes exactly
this recurrence along the free dimension at a measured ~2.14 ns/column and
is the only engine with the scan opcode on NeuronCore v3 (gpsimd is
rejected by the ISA engine check; running other engines concurrently
degrades the DVE by 1.4-4x via shared-SBUF contention, so a hybrid
pair-unroll loses). 16384 columns/partition/core -> ~35 us serial floor.

Layout: 512 lanes per core, packed 4 per partition, concatenated along the
free dim into one [128, 16384] stream. The host zeroes g[:, 0] of every
lane (it multiplies the zero initial state), so the scan state self-resets
at lane boundaries and the whole stream is scanned by chained
tensor_tensor_scan instructions (the carry passes through the previous
segment's last output column).

DMA: per-ring HWDGE throughput is descriptor-size bound (measured ~80 B/ns
at 512 B lines, ~130 at 1 KB, ~170 at 2 KB, ~230 at 4 KB, ~350 at 8 KB);
a [128, s] bf16 transfer has 128 descriptors of 2s bytes. So the body
segments are 4096 columns (8 KB lines) and the head segment is loaded as
four partition-sliced transfers ([32, 2048] each, 4 KB lines) so the
first scan can start early without paying the small-descriptor penalty.
g rides the sync ring, x the scalar ring, in scan order; stores alternate
between the rings (they queue behind the loads) and the final small store
is split across both rings to shorten the post-scan drain.
"""

import sys

import numpy as np

for _p in ("/opt/trn_rl_repo", "/opt/pypackages"):
    if _p not in sys.path:
        sys.path.append(_p)

import concourse.bacc as bacc
import concourse.mybir as mybir
from concourse.bass_utils import run_bass_kernel_spmd
from concourse.tile import TileContext

B, N, D = 4, 4096, 1024
N_CORES = 8
LANES = B * D                        # 4096 independent (b, d) lanes
LANES_PER_CORE = LANES // N_CORES    # 512
P = 128                              # SBUF partitions
LPP = LANES_PER_CORE // P            # 4 lanes per partition
NC = LPP * N                         # 16384 columns per partition

TRACE = False       # test harness sets True to capture a neuron-profile trace
USE_BF16 = True     # bf16 inputs: quantization ~2.6e-3 rel, halves load bytes
BF16_OUT = True     # bf16 output stores: halves store bytes
_result_info = {}   # exec_time_ns / trace path from the last run

import os as _os

_SEGS = [int(s) for s in _os.environ.get(
    "SEGS", "2048,4096,4096,4096,1536,512"
).split(",")]
assert sum(_SEGS) == NC
# Head segments (index < _PSPLIT_UPTO) load as 4 partition-sliced transfers
# so their descriptor size doubles twice (2s/4 bytes -> 2s bytes... a
# [32, s] slice has lines of 2s bytes like the full transfer, but only 32
# descriptors each; 4 concurrent transfers let all 16 DMA engines work the
# head at once).
_PSPLIT_UPTO = int(_os.environ.get("PSPLIT_UPTO", "1"))


def _build() -> bacc.Bacc:
    in_dt = mybir.dt.bfloat16
    out_dt = mybir.dt.bfloat16 if BF16_OUT else mybir.dt.float32
    nc = bacc.Bacc()
    gs = [
        nc.dram_tensor(f"g{k}", [P, seg], in_dt, kind="ExternalInput")
        for k, seg in enumerate(_SEGS)
    ]
    xs = [
        nc.dram_tensor(f"x{k}", [P, seg], in_dt, kind="ExternalInput")
        for k, seg in enumerate(_SEGS)
    ]
    os_ = [
        nc.dram_tensor(f"o{k}", [P, seg], out_dt, kind="ExternalOutput")
        for k, seg in enumerate(_SEGS)
    ]
    M = mybir.AluOpType.mult
    A = mybir.AluOpType.add
    with TileContext(nc) as tc:
        with tc.tile_pool(name="pool", bufs=1) as pool:
            gts = [pool.tile([P, s], in_dt, name=f"gt{k}") for k, s in enumerate(_SEGS)]
            xts = [pool.tile([P, s], in_dt, name=f"xt{k}") for k, s in enumerate(_SEGS)]
            ots = [pool.tile([P, s], out_dt, name=f"ot{k}") for k, s in enumerate(_SEGS)]
            # Loads in scan order: g on sync, x on scalar. Head segments are
            # partition-split 4-ways so the first columns land sooner.
            for k in range(len(_SEGS)):
                if k < _PSPLIT_UPTO:
                    for q in range(4):
                        rs = slice(q * 32, (q + 1) * 32)
                        nc.sync.dma_start(out=gts[k][rs, :], in_=gs[k][rs, :])
                        nc.scalar.dma_start(out=xts[k][rs, :], in_=xs[k][rs, :])
                else:
                    nc.sync.dma_start(out=gts[k][:, :], in_=gs[k][:, :])
                    nc.scalar.dma_start(out=xts[k][:, :], in_=xs[k][:, :])
            # Chained scans; carry crosses segment boundaries through the
            # previous segment's last output column (bf16 rounding there is
            # far inside the error budget). Stores alternate rings; the last
            # (small) store is split across both rings.
            prev = None
            last = len(_SEGS) - 1
            for k, seg in enumerate(_SEGS):
                init = 0.0 if prev is None else prev
                nc.vector.tensor_tensor_scan(
                    ots[k][:, :], gts[k][:, :], xts[k][:, :], init, M, A
                )
                prev = ots[k][:, seg - 1 : seg]
                if k == last:
                    h = seg // 2
                    nc.sync.dma_start(out=os_[k][:, 0:h], in_=ots[k][:, 0:h])
                    nc.scalar.dma_start(out=os_[k][:, h:seg], in_=ots[k][:, h:seg])
                elif k % 2 == 0:
                    nc.sync.dma_start(out=os_[k][:, :], in_=ots[k][:, :])
                else:
                    nc.scalar.dma_start(out=os_[k][:, :], in_=ots[k][:, :])
    nc.compile()
    return nc


def kernel(gates: np.ndarray, inputs: np.ndarray) -> np.ndarray:
    import ml_dtypes

    gates = np.asarray(gates, dtype=np.float32)
    inputs = np.asarray(inputs, dtype=np.float32)

    # Host-side shard: (B, N, D) -> lane-major (B*D, N); row b*D + d is the
    # contiguous time series of lane (b, d). The first gate of every lane
    # multiplies the zero initial state, so it is dead — zero it to make
    # the scan state reset at lane boundaries after concatenation.
    gt = np.ascontiguousarray(gates.transpose(0, 2, 1)).reshape(LANES, N)
    xt = np.ascontiguousarray(inputs.transpose(0, 2, 1)).reshape(LANES, N)
    gt[:, 0] = 0.0
    gt = gt.astype(ml_dtypes.bfloat16)
    xt = xt.astype(ml_dtypes.bfloat16)

    # Per core: [512, N] -> [LPP, P, N] -> [P, LPP, N] -> [P, NC]: partition
    # p holds lanes {base + p, base + P + p, ...} concatenated in time.
    bounds = np.cumsum([0] + _SEGS)
    in_maps = []
    for c in range(N_CORES):
        rows = slice(c * LANES_PER_CORE, (c + 1) * LANES_PER_CORE)
        gc = gt[rows].reshape(LPP, P, N).transpose(1, 0, 2).reshape(P, NC)
        xc = xt[rows].reshape(LPP, P, N).transpose(1, 0, 2).reshape(P, NC)
        m = {}
        for k in range(len(_SEGS)):
            sl = slice(bounds[k], bounds[k + 1])
            m[f"g{k}"] = np.ascontiguousarray(gc[:, sl])
            m[f"x{k}"] = np.ascontiguousarray(xc[:, sl])
        in_maps.append(m)

    nc = _build()
    res = run_bass_kernel_spmd(
        nc, in_maps, core_ids=list(range(N_CORES)), trace=TRACE
    )
    _result_info["exec_time_ns"] = res.exec_time_ns
    _result_info["mean_exec_time_ns"] = res.mean_exec_time_ns
    _result_info["profile_json"] = res.profile_json
    _result_info["trace"] = (
        res.instructions_and_trace[1] if res.instructions_and_trace else None
    )

    parts = []
    for c in range(N_CORES):
        oc = np.concatenate(
            [
                res.results[c][f"o{k}"].astype(np.float32, copy=False)
                for k in range(len(_SEGS))
            ],
            axis=1,
        )
        parts.append(
            oc.reshape(P, LPP, N).transpose(1, 0, 2).reshape(LANES_PER_CORE, N)
        )
    out_t = np.concatenate(parts, axis=0)  # (LANES, N)
    return np.ascontiguousarray(out_t.reshape(B, D, N).transpose(0, 2, 1))


# revision 12
# speedup vs baseline: 1.5959x; 1.0773x over previous
"""AssocScan Trainium2 kernel: out[:, t] = gates[:, t] * out[:, t-1] + inputs[:, t].

Strategy: the recurrence is independent per (b, d) lane (B*D = 4096 lanes,
N = 4096 steps). The DVE `tensor_tensor_scan` instruction computes exactly
this recurrence along the free dimension at a measured ~2.14 ns/column and
is the only engine with the scan opcode on NeuronCore v3 (gpsimd is
rejected by the ISA engine check; running other engines concurrently
degrades the DVE by 1.4-4x via shared-SBUF contention, so a hybrid
pair-unroll loses). 16384 columns/partition/core -> ~35 us serial floor.

Layout: 512 lanes per core, packed 4 per partition, concatenated along the
free dim into one [128, 16384] stream. The host zeroes g[:, 0] of every
lane (it multiplies the zero initial state), so the scan state self-resets
at lane boundaries and the whole stream is scanned by chained
tensor_tensor_scan instructions (the carry passes through the previous
segment's last output column).

DMA: per-ring HWDGE throughput is descriptor-size bound (measured ~80 B/ns
at 512 B lines, ~130 at 1 KB, ~170 at 2 KB, ~230 at 4 KB, ~350 at 8 KB);
a [128, s] bf16 transfer has 128 descriptors of 2s bytes. So the body
segments are 4096 columns (8 KB lines) and the head segment is loaded as
four partition-sliced transfers ([32, 2048] each, 4 KB lines) so the
first scan can start early without paying the small-descriptor penalty.
g rides the sync ring, x the scalar ring, in scan order; stores alternate
between the rings (they queue behind the loads) and the final small store
is split across both rings to shorten the post-scan drain.
"""

import sys

import numpy as np

for _p in ("/opt/trn_rl_repo", "/opt/pypackages"):
    if _p not in sys.path:
        sys.path.append(_p)

import concourse.bacc as bacc
import concourse.mybir as mybir
from concourse.bass_utils import run_bass_kernel_spmd
from concourse.tile import TileContext

B, N, D = 4, 4096, 1024
N_CORES = 8
LANES = B * D                        # 4096 independent (b, d) lanes
LANES_PER_CORE = LANES // N_CORES    # 512
P = 128                              # SBUF partitions
LPP = LANES_PER_CORE // P            # 4 lanes per partition
NC = LPP * N                         # 16384 columns per partition

TRACE = False       # test harness sets True to capture a neuron-profile trace
USE_BF16 = True     # bf16 inputs: quantization ~2.6e-3 rel, halves load bytes
BF16_OUT = True     # bf16 output stores: halves store bytes
_result_info = {}   # exec_time_ns / trace path from the last run

import os as _os

# All 8 cores run in lockstep, so loads see an HBM fair share of only
# ~270-300 B/ns per core while the scan consumes ~240 B/ns — delivery
# barely outpaces consumption. A steep (2x) geometric head therefore
# starves mid-stream; the ramp must grow no faster than the
# delivery/consumption ratio (~1.2-1.3x per segment).
_SEGS = [int(s) for s in _os.environ.get(
    "SEGS", "512,640,768,1024,1280,1536,2048,2560,2560,2048,1024,384"
).split(",")]
assert sum(_SEGS) == NC
# Head segments (index < _PSPLIT_UPTO) load as 4 partition-sliced transfers
# so their descriptor size doubles twice (2s/4 bytes -> 2s bytes... a
# [32, s] slice has lines of 2s bytes like the full transfer, but only 32
# descriptors each; 4 concurrent transfers let all 16 DMA engines work the
# head at once).
_PSPLIT_UPTO = int(_os.environ.get("PSPLIT_UPTO", "1"))


def _build() -> bacc.Bacc:
    in_dt = mybir.dt.bfloat16
    out_dt = mybir.dt.bfloat16 if BF16_OUT else mybir.dt.float32
    nc = bacc.Bacc()
    gs = [
        nc.dram_tensor(f"g{k}", [P, seg], in_dt, kind="ExternalInput")
        for k, seg in enumerate(_SEGS)
    ]
    xs = [
        nc.dram_tensor(f"x{k}", [P, seg], in_dt, kind="ExternalInput")
        for k, seg in enumerate(_SEGS)
    ]
    os_ = [
        nc.dram_tensor(f"o{k}", [P, seg], out_dt, kind="ExternalOutput")
        for k, seg in enumerate(_SEGS)
    ]
    M = mybir.AluOpType.mult
    A = mybir.AluOpType.add
    with TileContext(nc) as tc:
        with tc.tile_pool(name="pool", bufs=1) as pool:
            gts = [pool.tile([P, s], in_dt, name=f"gt{k}") for k, s in enumerate(_SEGS)]
            xts = [pool.tile([P, s], in_dt, name=f"xt{k}") for k, s in enumerate(_SEGS)]
            ots = [pool.tile([P, s], out_dt, name=f"ot{k}") for k, s in enumerate(_SEGS)]
            # Loads in scan order: g on sync, x on scalar. Head segments are
            # partition-split 4-ways so the first columns land sooner.
            for k in range(len(_SEGS)):
                if k < _PSPLIT_UPTO:
                    for q in range(4):
                        rs = slice(q * 32, (q + 1) * 32)
                        nc.sync.dma_start(out=gts[k][rs, :], in_=gs[k][rs, :])
                        nc.scalar.dma_start(out=xts[k][rs, :], in_=xs[k][rs, :])
                else:
                    nc.sync.dma_start(out=gts[k][:, :], in_=gs[k][:, :])
                    nc.scalar.dma_start(out=xts[k][:, :], in_=xs[k][:, :])
            # Chained scans; carry crosses segment boundaries through the
            # previous segment's last output column (bf16 rounding there is
            # far inside the error budget). Stores alternate rings; the last
            # (small) store is split across both rings.
            prev = None
            last = len(_SEGS) - 1
            for k, seg in enumerate(_SEGS):
                init = 0.0 if prev is None else prev
                nc.vector.tensor_tensor_scan(
                    ots[k][:, :], gts[k][:, :], xts[k][:, :], init, M, A
                )
                prev = ots[k][:, seg - 1 : seg]
                if k == last:
                    h = seg // 2
                    nc.sync.dma_start(out=os_[k][:, 0:h], in_=ots[k][:, 0:h])
                    nc.scalar.dma_start(out=os_[k][:, h:seg], in_=ots[k][:, h:seg])
                elif k % 2 == 0:
                    nc.sync.dma_start(out=os_[k][:, :], in_=ots[k][:, :])
                else:
                    nc.scalar.dma_start(out=os_[k][:, :], in_=ots[k][:, :])
    nc.compile()
    return nc


def kernel(gates: np.ndarray, inputs: np.ndarray) -> np.ndarray:
    import ml_dtypes

    gates = np.asarray(gates, dtype=np.float32)
    inputs = np.asarray(inputs, dtype=np.float32)

    # Host-side shard: (B, N, D) -> lane-major (B*D, N); row b*D + d is the
    # contiguous time series of lane (b, d). The first gate of every lane
    # multiplies the zero initial state, so it is dead — zero it to make
    # the scan state reset at lane boundaries after concatenation.
    gt = np.ascontiguousarray(gates.transpose(0, 2, 1)).reshape(LANES, N)
    xt = np.ascontiguousarray(inputs.transpose(0, 2, 1)).reshape(LANES, N)
    gt[:, 0] = 0.0
    gt = gt.astype(ml_dtypes.bfloat16)
    xt = xt.astype(ml_dtypes.bfloat16)

    # Per core: [512, N] -> [LPP, P, N] -> [P, LPP, N] -> [P, NC]: partition
    # p holds lanes {base + p, base + P + p, ...} concatenated in time.
    bounds = np.cumsum([0] + _SEGS)
    in_maps = []
    for c in range(N_CORES):
        rows = slice(c * LANES_PER_CORE, (c + 1) * LANES_PER_CORE)
        gc = gt[rows].reshape(LPP, P, N).transpose(1, 0, 2).reshape(P, NC)
        xc = xt[rows].reshape(LPP, P, N).transpose(1, 0, 2).reshape(P, NC)
        m = {}
        for k in range(len(_SEGS)):
            sl = slice(bounds[k], bounds[k + 1])
            m[f"g{k}"] = np.ascontiguousarray(gc[:, sl])
            m[f"x{k}"] = np.ascontiguousarray(xc[:, sl])
        in_maps.append(m)

    nc = _build()
    res = run_bass_kernel_spmd(
        nc, in_maps, core_ids=list(range(N_CORES)), trace=TRACE
    )
    _result_info["exec_time_ns"] = res.exec_time_ns
    _result_info["mean_exec_time_ns"] = res.mean_exec_time_ns
    _result_info["profile_json"] = res.profile_json
    _result_info["trace"] = (
        res.instructions_and_trace[1] if res.instructions_and_trace else None
    )

    parts = []
    for c in range(N_CORES):
        oc = np.concatenate(
            [
                res.results[c][f"o{k}"].astype(np.float32, copy=False)
                for k in range(len(_SEGS))
            ],
            axis=1,
        )
        parts.append(
            oc.reshape(P, LPP, N).transpose(1, 0, 2).reshape(LANES_PER_CORE, N)
        )
    out_t = np.concatenate(parts, axis=0)  # (LANES, N)
    return np.ascontiguousarray(out_t.reshape(B, D, N).transpose(0, 2, 1))


# revision 13
# speedup vs baseline: 1.6200x; 1.0152x over previous
"""AssocScan Trainium2 kernel: out[:, t] = gates[:, t] * out[:, t-1] + inputs[:, t].

Strategy: the recurrence is independent per (b, d) lane (B*D = 4096 lanes,
N = 4096 steps). The DVE `tensor_tensor_scan` instruction computes exactly
this recurrence along the free dimension at a measured ~2.14 ns/column and
is the only engine with the scan opcode on NeuronCore v3 (gpsimd is
rejected by the ISA engine check; running other engines concurrently
degrades the DVE by 1.4-4x via shared-SBUF contention, so a hybrid
pair-unroll loses). 16384 columns/partition/core -> ~35 us serial floor.

Layout: 512 lanes per core, packed 4 per partition, concatenated along the
free dim into one [128, 16384] stream. The host zeroes g[:, 0] of every
lane (it multiplies the zero initial state), so the scan state self-resets
at lane boundaries and the whole stream is scanned by chained
tensor_tensor_scan instructions (the carry passes through the previous
segment's last output column).

DMA: per-ring HWDGE throughput is descriptor-size bound (measured ~80 B/ns
at 512 B lines, ~130 at 1 KB, ~170 at 2 KB, ~230 at 4 KB, ~350 at 8 KB);
a [128, s] bf16 transfer has 128 descriptors of 2s bytes. So the body
segments are 4096 columns (8 KB lines) and the head segment is loaded as
four partition-sliced transfers ([32, 2048] each, 4 KB lines) so the
first scan can start early without paying the small-descriptor penalty.
g rides the sync ring, x the scalar ring, in scan order; stores alternate
between the rings (they queue behind the loads) and the final small store
is split across both rings to shorten the post-scan drain.
"""

import sys

import numpy as np

for _p in ("/opt/trn_rl_repo", "/opt/pypackages"):
    if _p not in sys.path:
        sys.path.append(_p)

import concourse.bacc as bacc
import concourse.mybir as mybir
from concourse.bass_utils import run_bass_kernel_spmd
from concourse.tile import TileContext

B, N, D = 4, 4096, 1024
N_CORES = 8
LANES = B * D                        # 4096 independent (b, d) lanes
LANES_PER_CORE = LANES // N_CORES    # 512
P = 128                              # SBUF partitions
LPP = LANES_PER_CORE // P            # 4 lanes per partition
NC = LPP * N                         # 16384 columns per partition

TRACE = False       # test harness sets True to capture a neuron-profile trace
USE_BF16 = True     # bf16 inputs: quantization ~2.6e-3 rel, halves load bytes
BF16_OUT = True     # bf16 output stores: halves store bytes
_result_info = {}   # exec_time_ns / trace path from the last run

import os as _os

# All 8 cores run in lockstep, so loads see an HBM fair share of only
# ~270-300 B/ns per core while the scan consumes ~240 B/ns — delivery
# barely outpaces consumption. A steep (2x) geometric head therefore
# starves mid-stream; the ramp must grow no faster than the
# delivery/consumption ratio (~1.2-1.3x per segment).
_SEGS = [int(s) for s in _os.environ.get(
    "SEGS", "512,640,768,1024,1280,1536,2048,2560,2560,2048,1024,384"
).split(",")]
assert sum(_SEGS) == NC
# Head segments (index < _PSPLIT_UPTO) load as 4 partition-sliced transfers
# so their descriptor size doubles twice (2s/4 bytes -> 2s bytes... a
# [32, s] slice has lines of 2s bytes like the full transfer, but only 32
# descriptors each; 4 concurrent transfers let all 16 DMA engines work the
# head at once).
_PSPLIT_UPTO = int(_os.environ.get("PSPLIT_UPTO", "0"))


def _build() -> bacc.Bacc:
    in_dt = mybir.dt.bfloat16
    out_dt = mybir.dt.bfloat16 if BF16_OUT else mybir.dt.float32
    nc = bacc.Bacc()
    gs = [
        nc.dram_tensor(f"g{k}", [P, seg], in_dt, kind="ExternalInput")
        for k, seg in enumerate(_SEGS)
    ]
    xs = [
        nc.dram_tensor(f"x{k}", [P, seg], in_dt, kind="ExternalInput")
        for k, seg in enumerate(_SEGS)
    ]
    os_ = [
        nc.dram_tensor(f"o{k}", [P, seg], out_dt, kind="ExternalOutput")
        for k, seg in enumerate(_SEGS)
    ]
    M = mybir.AluOpType.mult
    A = mybir.AluOpType.add
    with TileContext(nc) as tc:
        with tc.tile_pool(name="pool", bufs=1) as pool:
            gts = [pool.tile([P, s], in_dt, name=f"gt{k}") for k, s in enumerate(_SEGS)]
            xts = [pool.tile([P, s], in_dt, name=f"xt{k}") for k, s in enumerate(_SEGS)]
            ots = [pool.tile([P, s], out_dt, name=f"ot{k}") for k, s in enumerate(_SEGS)]
            # Loads in scan order: g on sync, x on scalar. Head segments are
            # partition-split 4-ways so the first columns land sooner.
            for k in range(len(_SEGS)):
                if k < _PSPLIT_UPTO:
                    for q in range(4):
                        rs = slice(q * 32, (q + 1) * 32)
                        nc.sync.dma_start(out=gts[k][rs, :], in_=gs[k][rs, :])
                        nc.scalar.dma_start(out=xts[k][rs, :], in_=xs[k][rs, :])
                else:
                    nc.sync.dma_start(out=gts[k][:, :], in_=gs[k][:, :])
                    nc.scalar.dma_start(out=xts[k][:, :], in_=xs[k][:, :])
            # Chained scans; carry crosses segment boundaries through the
            # previous segment's last output column (bf16 rounding there is
            # far inside the error budget). Stores alternate rings; the last
            # (small) store is split across both rings.
            prev = None
            last = len(_SEGS) - 1
            for k, seg in enumerate(_SEGS):
                init = 0.0 if prev is None else prev
                nc.vector.tensor_tensor_scan(
                    ots[k][:, :], gts[k][:, :], xts[k][:, :], init, M, A
                )
                prev = ots[k][:, seg - 1 : seg]
                if k == last:
                    h = seg // 2
                    nc.sync.dma_start(out=os_[k][:, 0:h], in_=ots[k][:, 0:h])
                    nc.scalar.dma_start(out=os_[k][:, h:seg], in_=ots[k][:, h:seg])
                elif k % 2 == 0:
                    nc.sync.dma_start(out=os_[k][:, :], in_=ots[k][:, :])
                else:
                    nc.scalar.dma_start(out=os_[k][:, :], in_=ots[k][:, :])
    nc.compile()
    return nc


def kernel(gates: np.ndarray, inputs: np.ndarray) -> np.ndarray:
    import ml_dtypes

    gates = np.asarray(gates, dtype=np.float32)
    inputs = np.asarray(inputs, dtype=np.float32)

    # Host-side shard: (B, N, D) -> lane-major (B*D, N); row b*D + d is the
    # contiguous time series of lane (b, d). The first gate of every lane
    # multiplies the zero initial state, so it is dead — zero it to make
    # the scan state reset at lane boundaries after concatenation.
    gt = np.ascontiguousarray(gates.transpose(0, 2, 1)).reshape(LANES, N)
    xt = np.ascontiguousarray(inputs.transpose(0, 2, 1)).reshape(LANES, N)
    gt[:, 0] = 0.0
    gt = gt.astype(ml_dtypes.bfloat16)
    xt = xt.astype(ml_dtypes.bfloat16)

    # Per core: [512, N] -> [LPP, P, N] -> [P, LPP, N] -> [P, NC]: partition
    # p holds lanes {base + p, base + P + p, ...} concatenated in time.
    bounds = np.cumsum([0] + _SEGS)
    in_maps = []
    for c in range(N_CORES):
        rows = slice(c * LANES_PER_CORE, (c + 1) * LANES_PER_CORE)
        gc = gt[rows].reshape(LPP, P, N).transpose(1, 0, 2).reshape(P, NC)
        xc = xt[rows].reshape(LPP, P, N).transpose(1, 0, 2).reshape(P, NC)
        m = {}
        for k in range(len(_SEGS)):
            sl = slice(bounds[k], bounds[k + 1])
            m[f"g{k}"] = np.ascontiguousarray(gc[:, sl])
            m[f"x{k}"] = np.ascontiguousarray(xc[:, sl])
        in_maps.append(m)

    nc = _build()
    res = run_bass_kernel_spmd(
        nc, in_maps, core_ids=list(range(N_CORES)), trace=TRACE
    )
    _result_info["exec_time_ns"] = res.exec_time_ns
    _result_info["mean_exec_time_ns"] = res.mean_exec_time_ns
    _result_info["profile_json"] = res.profile_json
    _result_info["trace"] = (
        res.instructions_and_trace[1] if res.instructions_and_trace else None
    )

    parts = []
    for c in range(N_CORES):
        oc = np.concatenate(
            [
                res.results[c][f"o{k}"].astype(np.float32, copy=False)
                for k in range(len(_SEGS))
            ],
            axis=1,
        )
        parts.append(
            oc.reshape(P, LPP, N).transpose(1, 0, 2).reshape(LANES_PER_CORE, N)
        )
    out_t = np.concatenate(parts, axis=0)  # (LANES, N)
    return np.ascontiguousarray(out_t.reshape(B, D, N).transpose(0, 2, 1))
